# revision 1
# baseline (speedup 1.0000x reference)
"""Contrastive loss (NCE softmax over a similarity square) on 8 Trainium2 cores.

Math (B=8192, D=512, T=0.1, r=0.1):
    z   = normalize(emb)                       # row L2
    s   = sum_b emb[b, :]
    v_b = r*s + (1-2r)*emb[b];  pos_b = (z_b . v_b)/||v_b||
    logits row b = [pos_b, raw[b,1:]]/T with raw = z@z.T, diag(raw) tweaks
    loss = mean_b( logsumexp(row_b) - pos_b/T )

Because the row-b fixups cancel, the per-row exp-sum reduces to
    S_b = sum_j exp(raw[b,j]/T) + exp(pos_b/T) - exp(raw[b,b]/T)
with raw[b,b] = ||z_b||^2 = 1, so exp(raw[b,b]/T) ~= e^10 (constant).
Logits are bounded by 1/T=10, so no max-subtraction is needed in fp32.

Sharding: data-parallel over rows. Each core gets the full emb (to build the
all-rows z as matmul rhs) plus its own 1024-row shard, computes its
1024x8192 slice of exp-sums and a partial loss sum; host adds 8 partials.

Per-core pipeline:
  A. own shard: normalize -> z_own (bf16), stage to DRAM, DMA-transpose back
     as zT_own [4][128,1024] (the matmul lhsT).
  B. full emb in 4 row-groups: normalize -> z (bf16) -> DRAM -> DMA-transpose
     to zT [4][128,2048] per group; 8 bf16 matmuls per psum tile
     (z_own^T . z^T slices), fused exp(10x)+row-sum on ACT per [128,1024].
     A parallel 64-matmul chain accumulates s = sum_b z_b*||emb_b|| in psum.
  C. pos path in row-major land (s broadcast via a K=1 fp32 matmul).
  D. S fixup + log + partial row-sum via two ones-matmuls -> [1,1] output.
"""

import math

import numpy as np

import concourse.bacc as bacc
import concourse.mybir as mybir
import concourse.tile as tile
from concourse.bass_utils import run_bass_kernel_spmd

F32 = mybir.dt.float32
BF16 = mybir.dt.bfloat16
AF = mybir.ActivationFunctionType
ALU = mybir.AluOpType
AX = mybir.AxisListType

B = 8192
D = 512
N_CORES = 8
OWN = B // N_CORES          # 1024 rows per core
P = 128                     # partitions
NT = B // P                 # 64 full-emb row tiles
NG = 4                      # row groups (transpose pipelining)
TPG = NT // NG              # 16 tiles per group
GR = B // NG                # 2048 rows per group
MT = OWN // P               # 8 own row tiles
KC = D // P                 # 4 contraction chunks
SCALE = 10.0                # 1/TEMPERATURE
RATIO = 0.1
E10 = float(math.exp(SCALE))


def _body(ctx, tc, out, emb_full, emb_own):
    nc = tc.nc

    pp = ctx.enter_context(tc.tile_pool(name="persist", bufs=1))
    dp = ctx.enter_context(tc.tile_pool(name="dram", bufs=1, space="DRAM"))
    ep = ctx.enter_context(tc.tile_pool(name="ep", bufs=20))
    zp = ctx.enter_context(tc.tile_pool(name="zp", bufs=4))
    scrp = ctx.enter_context(tc.tile_pool(name="scrp", bufs=1))
    up = ctx.enter_context(tc.tile_pool(name="up", bufs=2))
    esp = ctx.enter_context(tc.tile_pool(name="esp", bufs=2))
    psm = ctx.enter_context(tc.tile_pool(name="psm", bufs=3, space="PSUM"))
    pss = ctx.enter_context(tc.tile_pool(name="pss", bufs=1, space="PSUM"))
    psf = ctx.enter_context(tc.tile_pool(name="psf", bufs=1, space="PSUM"))

    # persistent tiles
    zT = [[pp.tile([P, GR], BF16, tag=f"zT_{k}_{g}", name=f"zT_{k}_{g}")
           for g in range(NG)] for k in range(KC)]
    zTo = [pp.tile([P, OWN], BF16, tag=f"zTo_{k}", name=f"zTo_{k}")
           for k in range(KC)]
    eo = [pp.tile([P, D], F32, tag=f"eo_{m}", name=f"eo_{m}")
          for m in range(MT)]
    zo = [pp.tile([P, D], BF16, tag=f"zo_{m}", name=f"zo_{m}")
          for m in range(MT)]
    zof = [pp.tile([P, D], F32, tag=f"zof_{m}", name=f"zof_{m}")
           for m in range(MT)]
    normbf = pp.tile([P, NT], BF16, tag="normbf", name="normbf")
    sqg = pp.tile([P, NT], F32, tag="sqg", name="sqg")
    lng = pp.tile([P, NT], F32, tag="lng", name="lng")
    invg = pp.tile([P, NT], F32, tag="invg", name="invg")
    scols = pp.tile([P, MT * 8], F32, tag="scols", name="scols")
    osq = pp.tile([P, MT], F32, tag="osq", name="osq")
    oln = pp.tile([P, MT], F32, tag="oln", name="oln")
    oinv = pp.tile([P, MT], F32, tag="oinv", name="oinv")
    vsq = pp.tile([P, MT], F32, tag="vsq", name="vsq")
    zv = pp.tile([P, MT], F32, tag="zv", name="zv")
    vln = pp.tile([P, MT], F32, tag="vln", name="vln")
    vninv = pp.tile([P, MT], F32, tag="vninv", name="vninv")
    possim = pp.tile([P, MT], F32, tag="possim", name="possim")
    pos10 = pp.tile([P, MT], F32, tag="pos10", name="pos10")
    epos = pp.tile([P, MT], F32, tag="epos", name="epos")
    stot = pp.tile([P, MT], F32, tag="stot", name="stot")
    sfix = pp.tile([P, MT], F32, tag="sfix", name="sfix")
    lg = pp.tile([P, MT], F32, tag="lg", name="lg")
    loss8 = pp.tile([P, MT], F32, tag="loss8", name="loss8")
    sbc = pp.tile([P, D], F32, tag="sbc", name="sbc")
    s01 = pp.tile([1, D], F32, tag="s01", name="s01")
    ones_row = pp.tile([1, P], F32, tag="ones_row", name="ones_row")
    ones_col = pp.tile([P, 1], F32, tag="ones_col", name="ones_col")
    ones8 = pp.tile([MT, 1], F32, tag="ones8", name="ones8")
    l8 = pp.tile([MT, 1], F32, tag="l8", name="l8")
    res = pp.tile([1, 1], F32, tag="res", name="res")

    zdr = [dp.tile([GR, D], BF16, tag=f"zdr_{g}", name=f"zdr_{g}")
           for g in range(NG)]
    zodr = dp.tile([OWN, D], BF16, tag="zodr", name="zodr")

    nc.vector.memset(ones_row, 1.0)
    nc.vector.memset(ones_col, 1.0)
    nc.vector.memset(ones8, 1.0)

    # ---- Phase A: own shard -> zT_own ----
    for m in range(MT):
        nc.sync.dma_start(eo[m], emb_own[m * P:(m + 1) * P, :])
    for m in range(MT):
        scr = scrp.tile([P, D], F32, tag="scr", name="scr")
        nc.vector.scalar_tensor_tensor(
            out=scr, in0=eo[m], scalar=1.0, in1=eo[m],
            op0=ALU.mult, op1=ALU.mult, accum_out=osq[:, m:m + 1])
    # inv_norm = exp(-0.5*ln(x)) keeps every ACT op in one table set
    nc.scalar.activation(out=oln, in_=osq, func=AF.Ln)
    nc.scalar.activation(out=oinv, in_=oln, func=AF.Exp, scale=-0.5)
    for m in range(MT):
        nc.vector.tensor_scalar_mul(zof[m], eo[m], oinv[:, m:m + 1])
        nc.vector.tensor_copy(out=zo[m], in_=zof[m])
        nc.sync.dma_start(zodr[m * P:(m + 1) * P, :], zo[m])
    for k in range(KC):
        nc.sync.dma_start_transpose(zTo[k], zodr[:, k * P:(k + 1) * P])

    # ---- Phase B: full emb, grouped, software-pipelined emission ----
    s_psum = pss.tile([1, D], F32, tag="s", name="s")

    def emit_norm(g):
        g0, g1 = g * TPG, (g + 1) * TPG
        for t in range(TPG):
            gt = g * TPG + t
            e = ep.tile([P, D], F32, tag="e", name="e")
            nc.sync.dma_start(e, emb_full[gt * P:(gt + 1) * P, :])
            scr = scrp.tile([P, D], F32, tag="scr", name="scr")
            nc.vector.scalar_tensor_tensor(
                out=scr, in0=e, scalar=1.0, in1=e,
                op0=ALU.mult, op1=ALU.mult, accum_out=sqg[:, gt:gt + 1])
            e_tiles.append(e)
        nc.scalar.activation(out=lng[:, g0:g1], in_=sqg[:, g0:g1], func=AF.Ln)
        nc.scalar.activation(out=invg[:, g0:g1], in_=lng[:, g0:g1],
                             func=AF.Exp, scale=-0.5)
        # norm = sq * inv_norm (bf16 weights for the s-colsum matmul)
        nc.vector.tensor_mul(normbf[:, g0:g1], sqg[:, g0:g1], invg[:, g0:g1])
        for t in range(TPG):
            gt = g * TPG + t
            e = e_tiles[gt]
            z = zp.tile([P, D], BF16, tag="z", name="z")
            nc.vector.tensor_scalar_mul(z, e, invg[:, gt:gt + 1])
            nc.sync.dma_start(zdr[g][t * P:(t + 1) * P, :], z)
            # s accumulation: sum_b z_b * norm_b = colsum of emb
            nc.tensor.matmul(
                s_psum, lhsT=normbf[:, gt:gt + 1], rhs=z,
                start=(gt == 0), stop=(gt == NT - 1), skip_group_check=True)

    def emit_trans(g):
        for k in range(KC):
            nc.sync.dma_start_transpose(
                zT[k][g], zdr[g][:, k * P:(k + 1) * P])

    def emit_main(g):
        for m in range(MT):
            for half in range(2):
                ps = psm.tile([P, 1024], F32, tag="ps", name="ps")
                for sub in range(2):
                    c0 = (half * 2 + sub) * 512
                    for k in range(KC):
                        nc.tensor.matmul(
                            ps[:, sub * 512:(sub + 1) * 512],
                            lhsT=zTo[k][:, m * P:(m + 1) * P],
                            rhs=zT[k][g][:, c0:c0 + 512],
                            start=(k == 0), stop=(k == KC - 1),
                            skip_group_check=True)
                es = esp.tile([P, 1024], BF16, tag="es", name="es")
                col = m * 8 + g * 2 + half
                nc.scalar.activation(
                    out=es, in_=ps, func=AF.Exp, scale=SCALE,
                    accum_out=scols[:, col:col + 1])

    e_tiles = []
    for g in range(NG):
        if g == 0:
            emit_norm(0)
            emit_norm(1)
        elif g + 1 < NG:
            emit_norm(g + 1)
        emit_trans(g)
        emit_main(g)

    # ---- Phase C: positive-pair path ----
    nc.vector.tensor_scalar_mul(s01, s_psum, RATIO)
    sb_psum = psf.tile([P, D], F32, tag="fin", name="ps_sbc")
    nc.tensor.matmul(sb_psum, lhsT=ones_row, rhs=s01, start=True, stop=True)
    nc.vector.tensor_copy(out=sbc, in_=sb_psum)
    for m in range(MT):
        u = up.tile([P, D], F32, tag="u", name="u")
        nc.vector.scalar_tensor_tensor(
            out=u, in0=eo[m], scalar=1.0 - 2.0 * RATIO, in1=sbc,
            op0=ALU.mult, op1=ALU.add)
        scr = scrp.tile([P, D], F32, tag="scr", name="scr")
        nc.vector.scalar_tensor_tensor(
            out=scr, in0=u, scalar=1.0, in1=u,
            op0=ALU.mult, op1=ALU.mult, accum_out=vsq[:, m:m + 1])
        scr2 = scrp.tile([P, D], F32, tag="scr", name="scr")
        nc.vector.scalar_tensor_tensor(
            out=scr2, in0=zof[m], scalar=1.0, in1=u,
            op0=ALU.mult, op1=ALU.mult, accum_out=zv[:, m:m + 1])
    nc.scalar.activation(out=vln, in_=vsq, func=AF.Ln)
    nc.scalar.activation(out=vninv, in_=vln, func=AF.Exp, scale=-0.5)
    nc.vector.tensor_mul(possim, zv, vninv)
    nc.vector.tensor_scalar_mul(pos10, possim, SCALE)
    nc.scalar.activation(out=epos, in_=pos10, func=AF.Exp)

    # ---- Phase D: finale ----
    nc.vector.tensor_reduce(
        stot, scols.rearrange("p (m r) -> p m r", r=8), axis=AX.X,
        op=ALU.add)
    nc.vector.scalar_tensor_tensor(
        out=sfix, in0=stot, scalar=-E10, in1=epos, op0=ALU.add, op1=ALU.add)
    nc.scalar.activation(out=lg, in_=sfix, func=AF.Ln)
    nc.vector.tensor_sub(loss8, lg, pos10)
    f1 = psf.tile([MT, 1], F32, tag="fin", name="ps_f1")
    nc.tensor.matmul(f1, lhsT=loss8, rhs=ones_col, start=True, stop=True)
    nc.vector.tensor_copy(out=l8, in_=f1)
    f2 = psf.tile([1, 1], F32, tag="fin", name="ps_f2")
    nc.tensor.matmul(f2, lhsT=l8, rhs=ones8, start=True, stop=True)
    nc.vector.tensor_copy(out=res, in_=f2)
    nc.sync.dma_start(out, res)


_NC_CACHE = None


def _build():
    global _NC_CACHE
    if _NC_CACHE is not None:
        return _NC_CACHE
    nc = bacc.Bacc(
        "TRN2",
        target_bir_lowering=False,
        debug=False,
        enable_asserts=False,
        num_devices=N_CORES,
    )
    emb_full = nc.dram_tensor("emb_full", [B, D], F32, kind="ExternalInput").ap()
    emb_own = nc.dram_tensor("emb_own", [OWN, D], F32, kind="ExternalInput").ap()
    out = nc.dram_tensor("out", [1, 1], F32, kind="ExternalOutput").ap()
    from contextlib import ExitStack

    with tile.TileContext(nc) as tc, ExitStack() as ctx:
        _body(ctx, tc, out, emb_full, emb_own)
    nc.compile()
    _NC_CACHE = nc
    return nc


def run(emb: np.ndarray, trace: bool = False):
    """Run the SPMD kernel; returns (loss, BassKernelResults)."""
    emb = np.ascontiguousarray(np.asarray(emb, dtype=np.float32))
    assert emb.shape == (B, D)
    nc = _build()
    in_maps = [
        {
            "emb_full": emb,
            "emb_own": emb[c * OWN:(c + 1) * OWN],
        }
        for c in range(N_CORES)
    ]
    results = run_bass_kernel_spmd(
        nc, in_maps, core_ids=list(range(N_CORES)), trace=trace)
    total = 0.0
    for c in range(N_CORES):
        total += float(results.results[c]["out"][0, 0])
    loss = np.float32(total / B)
    return loss, results


def kernel(emb: np.ndarray) -> np.ndarray:
    loss, _ = run(emb, trace=False)
    return loss


if __name__ == "__main__":
    rng = np.random.default_rng(0)
    x = rng.standard_normal((B, D), dtype=np.float32)
    print("loss:", kernel(x))



# revision 17
# speedup vs baseline: 1.2170x; 1.2170x over previous
"""Contrastive loss (NCE softmax over a similarity square) on 8 Trainium2 cores.

Math (B=8192, D=512, T=0.1, r=0.1):
    z   = normalize(emb)                       # row L2
    s   = sum_b emb[b, :]
    v_b = r*s + (1-2r)*emb[b];  pos_b = (z_b . v_b)/||v_b||
    loss = mean_b( log(S_b) - 10*pos_b )
    S_b = sum_j exp(10*raw[b,j]) + exp(10*pos_b) - e^10   (raw = z@z.T)

Sharding (v3): true data-parallel. Each core loads ONLY its own 1024-row
shard, normalizes it, transposes it (bf16 DMA transpose via DRAM), casts to
fp8e4 scaled by S=32, and AllGathers the fp8 zT blocks (0.5MB/rank -> 4MB).
The gathered layout [r][p][k][j] gives matmul-ready [K=128, N] tiles per
rank-block. Main loop: fp8 DoubleRow matmuls (2 k-subtiles per pass) into
[128,2048] psum, fused exp((10/S^2)*x) + row-sum accumulation on ACT.
The column-sum s = sum_b emb_b is computed per-shard via a tiny bf16
ones-matmul and AllReduced ([1,512] fp32). Host adds the 8 partial losses.
"""

import math

import numpy as np

import concourse.bacc as bacc
import concourse.mybir as mybir
import concourse.tile as tile
from concourse.bass_utils import run_bass_kernel_spmd

F32 = mybir.dt.float32
BF16 = mybir.dt.bfloat16
FP8 = mybir.dt.float8e4
AF = mybir.ActivationFunctionType
ALU = mybir.AluOpType
AX = mybir.AxisListType
DR = mybir.MatmulPerfMode.DoubleRow

B = 8192
D = 512
N_CORES = 8
OWN = B // N_CORES          # 1024 rows per core
P = 128                     # partitions
MT = OWN // P               # 8 own row tiles
KC = D // P                 # 4 contraction chunks of 128
NR = N_CORES                # 8 rank blocks of 1024 columns
SCALE = 10.0                # 1/TEMPERATURE
RATIO = 0.1
E10 = float(math.exp(SCALE))
S8 = 32.0                   # fp8 pre-scale; matmul result is S8^2 * sim
ESC = SCALE / (S8 * S8)     # exp scale folding the fp8 pre-scale back out


def _body(ctx, tc, out, emb_own):
    nc = tc.nc

    pp = ctx.enter_context(tc.tile_pool(name="persist", bufs=1))
    dp = ctx.enter_context(tc.tile_pool(name="dram", bufs=1, space="DRAM"))
    scrp = ctx.enter_context(tc.tile_pool(name="scrp", bufs=1))
    up = ctx.enter_context(tc.tile_pool(name="up", bufs=2))
    esp = ctx.enter_context(tc.tile_pool(name="esp", bufs=2))
    psm = ctx.enter_context(tc.tile_pool(name="psm", bufs=2, space="PSUM"))

    # persistent tiles
    eo = [pp.tile([P, D], F32, tag=f"eo_{m}", name=f"eo_{m}")
          for m in range(MT)]
    zof = [pp.tile([P, D], F32, tag=f"zof_{m}", name=f"zof_{m}")
           for m in range(MT)]
    zbf = [pp.tile([P, D], BF16, tag=f"zbf_{m}", name=f"zbf_{m}")
           for m in range(MT)]
    zTbf = [pp.tile([P, OWN], BF16, tag=f"zTbf_{k}", name=f"zTbf_{k}")
            for k in range(KC)]
    zT8 = pp.tile([P, KC * OWN], FP8, tag="zT8", name="zT8")
    zr = [pp.tile([P, KC * OWN], FP8, tag=f"zr_{r}", name=f"zr_{r}")
          for r in range(NR)]
    osq = pp.tile([P, MT], F32, tag="osq", name="osq")
    oln = pp.tile([P, MT], F32, tag="oln", name="oln")
    oinv = pp.tile([P, MT], F32, tag="oinv", name="oinv")
    sinv = pp.tile([P, MT], F32, tag="sinv", name="sinv")
    normbf = pp.tile([P, MT], BF16, tag="normbf", name="normbf")
    scols = pp.tile([P, MT * 4], F32, tag="scols", name="scols")
    s01 = pp.tile([1, D], F32, tag="s01", name="s01")
    s01r = pp.tile([1, D], F32, tag="s01r", name="s01r")
    sbc = pp.tile([P, D], F32, tag="sbc", name="sbc")
    vsq = pp.tile([P, MT], F32, tag="vsq", name="vsq")
    zv = pp.tile([P, MT], F32, tag="zv", name="zv")
    vln = pp.tile([P, MT], F32, tag="vln", name="vln")
    vninv = pp.tile([P, MT], F32, tag="vninv", name="vninv")
    possim = pp.tile([P, MT], F32, tag="possim", name="possim")
    pos10 = pp.tile([P, MT], F32, tag="pos10", name="pos10")
    epos = pp.tile([P, MT], F32, tag="epos", name="epos")
    stot = pp.tile([P, MT], F32, tag="stot", name="stot")
    sfix = pp.tile([P, MT], F32, tag="sfix", name="sfix")
    lg = pp.tile([P, MT], F32, tag="lg", name="lg")
    loss8 = pp.tile([P, MT], F32, tag="loss8", name="loss8")
    ones_row = pp.tile([1, P], F32, tag="ones_row", name="ones_row")
    ones_col = pp.tile([P, 1], F32, tag="ones_col", name="ones_col")
    ones8 = pp.tile([MT, 1], F32, tag="ones8", name="ones8")
    l8 = pp.tile([MT, 1], F32, tag="l8", name="l8")
    res = pp.tile([1, 1], F32, tag="res", name="res")

    zodr = dp.tile([OWN, D], BF16, tag="zodr", name="zodr")
    ag_in = dp.tile([P, KC * OWN], FP8, tag="ag_in", name="ag_in")
    ar_in = dp.tile([1, D], F32, tag="ar_in", name="ar_in")
    ar_out = dp.tile([1, D], F32, tag="ar_out", name="ar_out")
    ag_out = dp.tile([NR * P, KC * OWN], FP8, tag="ag_out", name="ag_out")

    nc.vector.memset(ones_row, 1.0)
    nc.vector.memset(ones_col, 1.0)
    nc.vector.memset(ones8, 1.0)

    # ---- Phase A: own shard -> zT8 (fp8, S8-scaled, [p][k][j] layout) ----
    for m in range(MT):
        eng = nc.sync if m % 2 == 0 else nc.scalar
        eng.dma_start(eo[m], emb_own[m * P:(m + 1) * P, :])
    for m in range(MT):
        scr = scrp.tile([P, D], F32, tag="scr", name="scr")
        nc.vector.scalar_tensor_tensor(
            out=scr, in0=eo[m], scalar=1.0, in1=eo[m],
            op0=ALU.mult, op1=ALU.mult, accum_out=osq[:, m:m + 1])
    # inv_norm = exp(-0.5*ln(x)); Ln+Exp stay within one ACT table set
    nc.scalar.activation(out=oln, in_=osq, func=AF.Ln)
    nc.scalar.activation(out=oinv, in_=oln, func=AF.Exp, scale=-0.5)
    nc.vector.tensor_scalar_mul(sinv, oinv, S8)
    # norm/S8 in bf16: lhsT for the s columns-sum matmul (s = sum_b emb_b)
    nc.vector.scalar_tensor_tensor(
        out=normbf, in0=osq, scalar=1.0 / S8, in1=oinv,
        op0=ALU.mult, op1=ALU.mult)
    # zodr writes and transposes share the sync queue: FIFO order is the
    # only guaranteed DRAM write->transpose-read ordering.
    for m in range(MT):
        nc.vector.tensor_scalar_mul(zbf[m], eo[m], sinv[:, m:m + 1])
        nc.sync.dma_start(zodr[m * P:(m + 1) * P, :], zbf[m])
    for m in range(MT):
        nc.vector.tensor_scalar_mul(zof[m], eo[m], oinv[:, m:m + 1])
    for k in range(KC):
        nc.sync.dma_start_transpose(zTbf[k], zodr[:, k * P:(k + 1) * P])
        nc.vector.tensor_copy(out=zT8[:, k * OWN:(k + 1) * OWN],
                              in_=zTbf[k])
    nc.gpsimd.dma_start(ag_in[:], zT8)

    # s partial: sum_{own b} emb_b = sum_b (norm_b/S8) * (S8*z_b)  [bf16]
    s_ps = psm.tile([P, 2048], F32, tag="ps", name="ps_s")
    for m in range(MT):
        nc.tensor.matmul(s_ps[0:1, 0:D], lhsT=normbf[:, m:m + 1],
                         rhs=zbf[m], start=(m == 0), stop=(m == MT - 1),
                         skip_group_check=True)
    nc.vector.tensor_copy(out=s01, in_=s_ps[0:1, 0:D])
    nc.sync.dma_start(ar_in[:], s01)

    # ---- Collectives (gpsimd queue order: AG first, then AR) ----
    nc.gpsimd.collective_compute(
        "AllGather", ALU.bypass, replica_groups=[list(range(N_CORES))],
        ins=[ag_in.opt()], outs=[ag_out.opt()])
    nc.gpsimd.collective_compute(
        "AllReduce", ALU.add, replica_groups=[list(range(N_CORES))],
        ins=[ar_in.opt()], outs=[ar_out.opt()])

    # zr loads on the gpsimd queue: FIFO-ordered after the AllGather
    for r in range(NR):
        nc.gpsimd.dma_start(zr[r], ag_out[r * P:(r + 1) * P, :])

    # ---- Main loop: 8192x8192/8 similarity slice, exp-sum fused ----
    zT8v = zT8.rearrange("p (k j) -> p k j", k=KC)
    zrv = [zr[r].rearrange("p (k j) -> p k j", k=KC) for r in range(NR)]

    def emit_pos_dve():
        # v = (1-2r)*emb + r*s (fp32, row-major); DVE-only, overlaps main
        nc.vector.tensor_scalar_mul(s01r, ar_dst, RATIO)
        ps_b = psm.tile([P, 2048], F32, tag="ps", name="ps_bc")
        nc.tensor.matmul(ps_b[:, 0:D], lhsT=ones_row, rhs=s01r,
                         start=True, stop=True, skip_group_check=True)
        nc.vector.tensor_copy(out=sbc, in_=ps_b[:, 0:D])
        for m in range(MT):
            u = up.tile([P, D], F32, tag="u", name="u")
            nc.vector.scalar_tensor_tensor(
                out=u, in0=eo[m], scalar=1.0 - 2.0 * RATIO, in1=sbc,
                op0=ALU.mult, op1=ALU.add)
            scr = scrp.tile([P, D], F32, tag="scr", name="scr")
            nc.vector.scalar_tensor_tensor(
                out=scr, in0=u, scalar=1.0, in1=u,
                op0=ALU.mult, op1=ALU.mult, accum_out=vsq[:, m:m + 1])
            scr2 = scrp.tile([P, D], F32, tag="scr", name="scr")
            nc.vector.scalar_tensor_tensor(
                out=scr2, in0=zof[m], scalar=1.0, in1=u,
                op0=ALU.mult, op1=ALU.mult, accum_out=zv[:, m:m + 1])

    ar_dst = pp.tile([1, D], F32, tag="ar_dst", name="ar_dst")
    for rp in range(NR // 2):
        for m in range(MT):
            ps = psm.tile([P, 2048], F32, tag="ps", name="ps")
            for q in range(4):              # 4 x 512-col chunks (2 r-blocks)
                r = rp * 2 + q // 2
                c0 = (q % 2) * 512
                for kg in range(2):
                    nc.tensor.matmul(
                        ps[:, q * 512:(q + 1) * 512],
                        lhsT=zT8v[:, 2 * kg:2 * kg + 2, m * P:(m + 1) * P],
                        rhs=zrv[r][:, 2 * kg:2 * kg + 2, c0:c0 + 512],
                        start=(kg == 0), stop=(kg == 1),
                        perf_mode=DR, skip_group_check=True)
            es = esp.tile([P, 2048], BF16, tag="es", name="es")
            nc.scalar.activation(
                out=es, in_=ps, func=AF.Exp, scale=ESC,
                accum_out=scols[:, m * 4 + rp:m * 4 + rp + 1])
        if rp == 1:
            # AR result has landed by now; pos-path DVE overlaps the main loop
            nc.gpsimd.dma_start(ar_dst, ar_out[:])
            emit_pos_dve()

    # ---- Finale (ACT table switches confined to the tail) ----
    nc.scalar.activation(out=vln, in_=vsq, func=AF.Ln)
    nc.scalar.activation(out=vninv, in_=vln, func=AF.Exp, scale=-0.5)
    nc.vector.tensor_mul(possim, zv, vninv)
    nc.vector.tensor_scalar_mul(pos10, possim, SCALE)
    nc.scalar.activation(out=epos, in_=pos10, func=AF.Exp)
    nc.vector.tensor_reduce(
        stot, scols.rearrange("p (m g) -> p m g", g=4), axis=AX.X,
        op=ALU.add)
    nc.vector.scalar_tensor_tensor(
        out=sfix, in0=stot, scalar=-E10, in1=epos, op0=ALU.add, op1=ALU.add)
    nc.scalar.activation(out=lg, in_=sfix, func=AF.Ln)
    nc.vector.tensor_sub(loss8, lg, pos10)
    f1 = psm.tile([P, 2048], F32, tag="ps", name="ps_f1")
    nc.tensor.matmul(f1[0:MT, 0:1], lhsT=loss8, rhs=ones_col,
                     start=True, stop=True, skip_group_check=True)
    nc.vector.tensor_copy(out=l8, in_=f1[0:MT, 0:1])
    f2 = psm.tile([P, 2048], F32, tag="ps", name="ps_f2")
    nc.tensor.matmul(f2[0:1, 0:1], lhsT=l8, rhs=ones8,
                     start=True, stop=True, skip_group_check=True)
    nc.vector.tensor_copy(out=res, in_=f2[0:1, 0:1])
    nc.sync.dma_start(out, res)

    # debug taps (row 0 of each stage)
    dbg = tc.nc.dram_tensor("dbg", [1, 64], F32, kind="ExternalOutput").ap()
    dbf = pp.tile([1, 64], F32, tag="dbf", name="dbf")
    zr0f = pp.tile([1, 8], F32, tag="zr0f", name="zr0f")
    nc.vector.tensor_copy(out=zr0f, in_=zr[0][0:1, 0:8])
    nc.vector.tensor_copy(out=dbf[:, 0:8], in_=stot[0:1, :])
    nc.vector.tensor_copy(out=dbf[:, 8:16], in_=epos[0:1, :])
    nc.vector.tensor_copy(out=dbf[:, 16:24], in_=pos10[0:1, :])
    nc.vector.tensor_copy(out=dbf[:, 24:32], in_=scols[0:1, 0:8])
    nc.vector.tensor_copy(out=dbf[:, 32:40], in_=s01r[0:1, 0:8])
    nc.vector.tensor_copy(out=dbf[:, 40:48], in_=zr0f)
    nc.vector.tensor_copy(out=dbf[:, 48:56], in_=sfix[0:1, :])
    nc.vector.tensor_copy(out=dbf[:, 56:64], in_=osq[0:1, :])
    nc.sync.dma_start(dbg, dbf)
    dbg2 = tc.nc.dram_tensor("dbg2", [P, 40], F32, kind="ExternalOutput").ap()
    dbf2 = pp.tile([P, 40], F32, tag="dbf2", name="dbf2")
    nc.vector.tensor_copy(out=dbf2[:, 0:8], in_=stot)
    nc.vector.tensor_copy(out=dbf2[:, 8:16], in_=sfix)
    nc.vector.tensor_copy(out=dbf2[:, 16:24], in_=pos10)
    nc.vector.tensor_copy(out=dbf2[:, 24:32], in_=vsq)
    nc.vector.tensor_copy(out=dbf2[:, 32:40], in_=loss8)
    nc.sync.dma_start(dbg2, dbf2)
    dbg3 = tc.nc.dram_tensor("dbg3", [P, 16], F32, kind="ExternalOutput").ap()
    dbf3 = pp.tile([P, 16], F32, tag="dbf3", name="dbf3")
    for k in range(KC):
        nc.vector.tensor_copy(out=dbf3[:, k:k + 1],
                              in_=zT8[:, k * OWN + 129:k * OWN + 130])
        nc.vector.tensor_copy(out=dbf3[:, 4 + k:5 + k],
                              in_=zr[0][:, k * OWN + 129:k * OWN + 130])
        nc.vector.tensor_copy(out=dbf3[:, 8 + k:9 + k],
                              in_=zT8[:, k * OWN:k * OWN + 1])
        nc.vector.tensor_copy(out=dbf3[:, 12 + k:13 + k],
                              in_=zr[0][:, k * OWN:k * OWN + 1])
    nc.sync.dma_start(dbg3, dbf3)


_NC_CACHE = None


def _build():
    global _NC_CACHE
    if _NC_CACHE is not None:
        return _NC_CACHE
    nc = bacc.Bacc(
        "TRN2",
        target_bir_lowering=False,
        debug=False,
        enable_asserts=False,
        num_devices=N_CORES,
    )
    emb_own = nc.dram_tensor("emb_own", [OWN, D], F32, kind="ExternalInput").ap()
    out = nc.dram_tensor("out", [1, 1], F32, kind="ExternalOutput").ap()
    from contextlib import ExitStack

    with tile.TileContext(nc) as tc, ExitStack() as ctx:
        _body(ctx, tc, out, emb_own)
    nc.compile()
    _NC_CACHE = nc
    return nc


def run(emb: np.ndarray, trace: bool = False):
    """Run the SPMD kernel; returns (loss, BassKernelResults)."""
    emb = np.ascontiguousarray(np.asarray(emb, dtype=np.float32))
    assert emb.shape == (B, D)
    nc = _build()
    in_maps = [
        {"emb_own": emb[c * OWN:(c + 1) * OWN]}
        for c in range(N_CORES)
    ]
    results = run_bass_kernel_spmd(
        nc, in_maps, core_ids=list(range(N_CORES)), trace=trace)
    total = 0.0
    for c in range(N_CORES):
        total += float(results.results[c]["out"][0, 0])
    loss = np.float32(total / B)
    if "dbg" in results.results[0]:
        d = results.results[0]["dbg"][0]
        for i, nm in enumerate(["stot", "epos", "pos10", "scols",
                                "s01r", "zr0", "sfix", "osq"]):
            print(f"dbg {nm}: {d[i * 8:(i + 1) * 8]}")
    return loss, results


def kernel(emb: np.ndarray) -> np.ndarray:
    loss, _ = run(emb, trace=False)
    return loss


if __name__ == "__main__":
    rng = np.random.default_rng(0)
    x = rng.standard_normal((B, D), dtype=np.float32)
    print("loss:", kernel(x))


# revision 20
# speedup vs baseline: 1.2429x; 1.0212x over previous
"""Contrastive loss (NCE softmax over a similarity square) on 8 Trainium2 cores.

Math (B=8192, D=512, T=0.1, r=0.1):
    z   = normalize(emb)                       # row L2
    s   = sum_b emb[b, :]
    v_b = r*s + (1-2r)*emb[b];  pos_b = (z_b . v_b)/||v_b||
    loss = mean_b( log(S_b) - 10*pos_b )
    S_b = sum_j exp(10*raw[b,j]) + exp(10*pos_b) - e^10   (raw = z@z.T)

Sharding (v3): true data-parallel. Each core loads ONLY its own 1024-row
shard, normalizes it, transposes it (bf16 DMA transpose via DRAM), casts to
fp8e4 scaled by S=32, and AllGathers the fp8 zT blocks (0.5MB/rank -> 4MB).
The gathered layout [r][p][k][j] gives matmul-ready [K=128, N] tiles per
rank-block. Main loop: fp8 DoubleRow matmuls (2 k-subtiles per pass) into
[128,2048] psum, fused exp((10/S^2)*x) + row-sum accumulation on ACT.
The column-sum s = sum_b emb_b is computed per-shard via a tiny bf16
ones-matmul and AllReduced ([1,512] fp32). Host adds the 8 partial losses.
"""

import math

import numpy as np

import concourse.bacc as bacc
import concourse.mybir as mybir
import concourse.tile as tile
from concourse.bass_utils import run_bass_kernel_spmd

F32 = mybir.dt.float32
BF16 = mybir.dt.bfloat16
FP8 = mybir.dt.float8e4
AF = mybir.ActivationFunctionType
ALU = mybir.AluOpType
AX = mybir.AxisListType
DR = mybir.MatmulPerfMode.DoubleRow

B = 8192
D = 512
N_CORES = 8
OWN = B // N_CORES          # 1024 rows per core
P = 128                     # partitions
MT = OWN // P               # 8 own row tiles
KC = D // P                 # 4 contraction chunks of 128
NR = N_CORES                # 8 rank blocks of 1024 columns
SCALE = 10.0                # 1/TEMPERATURE
RATIO = 0.1
E10 = float(math.exp(SCALE))
S8 = 32.0                   # fp8 pre-scale; matmul result is S8^2 * sim
ESC = SCALE / (S8 * S8)     # exp scale folding the fp8 pre-scale back out


def _body(ctx, tc, out, emb_own):
    nc = tc.nc

    pp = ctx.enter_context(tc.tile_pool(name="persist", bufs=1))
    dp = ctx.enter_context(tc.tile_pool(name="dram", bufs=1, space="DRAM"))
    scrp = ctx.enter_context(tc.tile_pool(name="scrp", bufs=1))
    up = ctx.enter_context(tc.tile_pool(name="up", bufs=2))
    esp = ctx.enter_context(tc.tile_pool(name="esp", bufs=2))
    psm = ctx.enter_context(tc.tile_pool(name="psm", bufs=2, space="PSUM"))

    # persistent tiles
    eo = [pp.tile([P, D], F32, tag=f"eo_{m}", name=f"eo_{m}")
          for m in range(MT)]
    zof = [pp.tile([P, D], F32, tag=f"zof_{m}", name=f"zof_{m}")
           for m in range(MT)]
    zbf = [pp.tile([P, D], BF16, tag=f"zbf_{m}", name=f"zbf_{m}")
           for m in range(MT)]
    zTbf = [pp.tile([P, OWN], BF16, tag=f"zTbf_{k}", name=f"zTbf_{k}")
            for k in range(KC)]
    zT8 = pp.tile([P, KC * OWN], FP8, tag="zT8", name="zT8")
    zr = [pp.tile([P, KC * OWN], FP8, tag=f"zr_{r}", name=f"zr_{r}")
          for r in range(NR)]
    osq = pp.tile([P, MT], F32, tag="osq", name="osq")
    oln = pp.tile([P, MT], F32, tag="oln", name="oln")
    oinv = pp.tile([P, MT], F32, tag="oinv", name="oinv")
    sinv = pp.tile([P, MT], F32, tag="sinv", name="sinv")
    normbf = pp.tile([P, MT], BF16, tag="normbf", name="normbf")
    scols = pp.tile([P, MT * 4], F32, tag="scols", name="scols")
    s01 = pp.tile([1, D], F32, tag="s01", name="s01")
    s01r = pp.tile([1, D], F32, tag="s01r", name="s01r")
    sbc = pp.tile([P, D], F32, tag="sbc", name="sbc")
    vsq = pp.tile([P, MT], F32, tag="vsq", name="vsq")
    zv = pp.tile([P, MT], F32, tag="zv", name="zv")
    vln = pp.tile([P, MT], F32, tag="vln", name="vln")
    vninv = pp.tile([P, MT], F32, tag="vninv", name="vninv")
    possim = pp.tile([P, MT], F32, tag="possim", name="possim")
    pos10 = pp.tile([P, MT], F32, tag="pos10", name="pos10")
    epos = pp.tile([P, MT], F32, tag="epos", name="epos")
    stot = pp.tile([P, MT], F32, tag="stot", name="stot")
    sfix = pp.tile([P, MT], F32, tag="sfix", name="sfix")
    lg = pp.tile([P, MT], F32, tag="lg", name="lg")
    loss8 = pp.tile([P, MT], F32, tag="loss8", name="loss8")
    ones_row = pp.tile([1, P], F32, tag="ones_row", name="ones_row")
    ones_col = pp.tile([P, 1], F32, tag="ones_col", name="ones_col")
    ones8 = pp.tile([MT, 1], F32, tag="ones8", name="ones8")
    l8 = pp.tile([MT, 1], F32, tag="l8", name="l8")
    res = pp.tile([1, 1], F32, tag="res", name="res")

    zodr = dp.tile([OWN, D], BF16, tag="zodr", name="zodr")
    ag_in = dp.tile([P, KC * OWN], FP8, tag="ag_in", name="ag_in")
    ar_in = dp.tile([1, D], F32, tag="ar_in", name="ar_in")
    ar_out = dp.tile([1, D], F32, tag="ar_out", name="ar_out")
    ag_out = dp.tile([NR * P, KC * OWN], FP8, tag="ag_out", name="ag_out")

    nc.vector.memset(ones_row, 1.0)
    nc.vector.memset(ones_col, 1.0)
    nc.vector.memset(ones8, 1.0)

    # ---- Phase A: own shard -> zT8 (fp8, S8-scaled, [p][k][j] layout) ----
    for m in range(MT):
        eng = nc.sync if m % 2 == 0 else nc.scalar
        eng.dma_start(eo[m], emb_own[m * P:(m + 1) * P, :])
    for m in range(MT):
        scr = scrp.tile([P, D], F32, tag="scr", name="scr")
        nc.vector.scalar_tensor_tensor(
            out=scr, in0=eo[m], scalar=1.0, in1=eo[m],
            op0=ALU.mult, op1=ALU.mult, accum_out=osq[:, m:m + 1])
    # inv_norm = exp(-0.5*ln(x)); Ln+Exp stay within one ACT table set
    nc.scalar.activation(out=oln, in_=osq, func=AF.Ln)
    nc.scalar.activation(out=oinv, in_=oln, func=AF.Exp, scale=-0.5)
    nc.vector.tensor_scalar_mul(sinv, oinv, S8)
    # norm/S8 in bf16: lhsT for the s columns-sum matmul (s = sum_b emb_b)
    nc.vector.scalar_tensor_tensor(
        out=normbf, in0=osq, scalar=1.0 / S8, in1=oinv,
        op0=ALU.mult, op1=ALU.mult)
    # zodr writes and transposes share the sync queue: FIFO order is the
    # only guaranteed DRAM write->transpose-read ordering.
    for m in range(MT):
        nc.vector.tensor_scalar_mul(zbf[m], eo[m], sinv[:, m:m + 1])
        nc.sync.dma_start(zodr[m * P:(m + 1) * P, :], zbf[m])
    for m in range(MT):
        nc.vector.tensor_scalar_mul(zof[m], eo[m], oinv[:, m:m + 1])
    for k in range(KC):
        nc.sync.dma_start_transpose(zTbf[k], zodr[:, k * P:(k + 1) * P])
        nc.vector.tensor_copy(out=zT8[:, k * OWN:(k + 1) * OWN],
                              in_=zTbf[k])
    # sync queue: FIFO after the transposes; the AG's input-ready semaphore
    # is cross-queue tracked (proven by the ar_in path)
    nc.sync.dma_start(ag_in[:], zT8)

    # s partial: sum_{own b} emb_b = sum_b (norm_b/S8) * (S8*z_b)  [bf16]
    s_ps = psm.tile([P, 2048], F32, tag="ps", name="ps_s")
    for m in range(MT):
        nc.tensor.matmul(s_ps[0:1, 0:D], lhsT=normbf[:, m:m + 1],
                         rhs=zbf[m], start=(m == 0), stop=(m == MT - 1),
                         skip_group_check=True)
    nc.vector.tensor_copy(out=s01, in_=s_ps[0:1, 0:D])
    nc.sync.dma_start(ar_in[:], s01)

    # ---- Collectives (gpsimd queue order: AG first, then AR) ----
    nc.gpsimd.collective_compute(
        "AllGather", ALU.bypass, replica_groups=[list(range(N_CORES))],
        ins=[ag_in.opt()], outs=[ag_out.opt()])
    nc.gpsimd.collective_compute(
        "AllReduce", ALU.add, replica_groups=[list(range(N_CORES))],
        ins=[ar_in.opt()], outs=[ar_out.opt()])

    # zr loads on the gpsimd queue: FIFO-ordered after the AllGather
    for r in range(NR):
        nc.gpsimd.dma_start(zr[r], ag_out[r * P:(r + 1) * P, :])

    # ---- Main loop: 8192x8192/8 similarity slice, exp-sum fused ----
    zT8v = zT8.rearrange("p (k j) -> p k j", k=KC)
    zrv = [zr[r].rearrange("p (k j) -> p k j", k=KC) for r in range(NR)]

    def emit_pos_dve():
        # v = (1-2r)*emb + r*s (fp32, row-major); DVE-only, overlaps main
        nc.vector.tensor_scalar_mul(s01r, ar_dst, RATIO)
        ps_b = psm.tile([P, 2048], F32, tag="ps", name="ps_bc")
        nc.tensor.matmul(ps_b[:, 0:D], lhsT=ones_row, rhs=s01r,
                         start=True, stop=True, skip_group_check=True)
        nc.vector.tensor_copy(out=sbc, in_=ps_b[:, 0:D])
        for m in range(MT):
            u = up.tile([P, D], F32, tag="u", name="u")
            nc.vector.scalar_tensor_tensor(
                out=u, in0=eo[m], scalar=1.0 - 2.0 * RATIO, in1=sbc,
                op0=ALU.mult, op1=ALU.add)
            scr = scrp.tile([P, D], F32, tag="scr", name="scr")
            nc.vector.scalar_tensor_tensor(
                out=scr, in0=u, scalar=1.0, in1=u,
                op0=ALU.mult, op1=ALU.mult, accum_out=vsq[:, m:m + 1])
            scr2 = scrp.tile([P, D], F32, tag="scr", name="scr")
            nc.vector.scalar_tensor_tensor(
                out=scr2, in0=zof[m], scalar=1.0, in1=u,
                op0=ALU.mult, op1=ALU.mult, accum_out=zv[:, m:m + 1])

    ar_dst = pp.tile([1, D], F32, tag="ar_dst", name="ar_dst")
    for rp in range(NR // 2):
        for m in range(MT):
            ps = psm.tile([P, 2048], F32, tag="ps", name="ps")
            for q in range(4):              # 4 x 512-col chunks (2 r-blocks)
                r = rp * 2 + q // 2
                c0 = (q % 2) * 512
                for kg in range(2):
                    nc.tensor.matmul(
                        ps[:, q * 512:(q + 1) * 512],
                        lhsT=zT8v[:, 2 * kg:2 * kg + 2, m * P:(m + 1) * P],
                        rhs=zrv[r][:, 2 * kg:2 * kg + 2, c0:c0 + 512],
                        start=(kg == 0), stop=(kg == 1),
                        perf_mode=DR, skip_group_check=True)
            es = esp.tile([P, 2048], BF16, tag="es", name="es")
            nc.scalar.activation(
                out=es, in_=ps, func=AF.Exp, scale=ESC,
                accum_out=scols[:, m * 4 + rp:m * 4 + rp + 1])
        if rp == 1:
            # AR result has landed by now; pos-path DVE overlaps the main loop
            nc.gpsimd.dma_start(ar_dst, ar_out[:])
            emit_pos_dve()

    # ---- Finale (ACT table switches confined to the tail) ----
    nc.scalar.activation(out=vln, in_=vsq, func=AF.Ln)
    nc.scalar.activation(out=vninv, in_=vln, func=AF.Exp, scale=-0.5)
    nc.vector.tensor_mul(possim, zv, vninv)
    nc.vector.tensor_scalar_mul(pos10, possim, SCALE)
    nc.scalar.activation(out=epos, in_=pos10, func=AF.Exp)
    nc.vector.tensor_reduce(
        stot, scols.rearrange("p (m g) -> p m g", g=4), axis=AX.X,
        op=ALU.add)
    nc.vector.scalar_tensor_tensor(
        out=sfix, in0=stot, scalar=-E10, in1=epos, op0=ALU.add, op1=ALU.add)
    nc.scalar.activation(out=lg, in_=sfix, func=AF.Ln)
    nc.vector.tensor_sub(loss8, lg, pos10)
    f1 = psm.tile([P, 2048], F32, tag="ps", name="ps_f1")
    nc.tensor.matmul(f1[0:MT, 0:1], lhsT=loss8, rhs=ones_col,
                     start=True, stop=True, skip_group_check=True)
    nc.vector.tensor_copy(out=l8, in_=f1[0:MT, 0:1])
    f2 = psm.tile([P, 2048], F32, tag="ps", name="ps_f2")
    nc.tensor.matmul(f2[0:1, 0:1], lhsT=l8, rhs=ones8,
                     start=True, stop=True, skip_group_check=True)
    nc.vector.tensor_copy(out=res, in_=f2[0:1, 0:1])
    nc.sync.dma_start(out, res)


_NC_CACHE = None


def _build():
    global _NC_CACHE
    if _NC_CACHE is not None:
        return _NC_CACHE
    nc = bacc.Bacc(
        "TRN2",
        target_bir_lowering=False,
        debug=False,
        enable_asserts=False,
        num_devices=N_CORES,
    )
    emb_own = nc.dram_tensor("emb_own", [OWN, D], F32, kind="ExternalInput").ap()
    out = nc.dram_tensor("out", [1, 1], F32, kind="ExternalOutput").ap()
    from contextlib import ExitStack

    with tile.TileContext(nc) as tc, ExitStack() as ctx:
        _body(ctx, tc, out, emb_own)
    nc.compile()
    _NC_CACHE = nc
    return nc


def run(emb: np.ndarray, trace: bool = False):
    """Run the SPMD kernel; returns (loss, BassKernelResults)."""
    emb = np.ascontiguousarray(np.asarray(emb, dtype=np.float32))
    assert emb.shape == (B, D)
    nc = _build()
    in_maps = [
        {"emb_own": emb[c * OWN:(c + 1) * OWN]}
        for c in range(N_CORES)
    ]
    results = run_bass_kernel_spmd(
        nc, in_maps, core_ids=list(range(N_CORES)), trace=trace)
    total = 0.0
    for c in range(N_CORES):
        total += float(results.results[c]["out"][0, 0])
    loss = np.float32(total / B)
    return loss, results


def kernel(emb: np.ndarray) -> np.ndarray:
    loss, _ = run(emb, trace=False)
    return loss


if __name__ == "__main__":
    rng = np.random.default_rng(0)
    x = rng.standard_normal((B, D), dtype=np.float32)
    print("loss:", kernel(x))


# revision 27
# speedup vs baseline: 1.3614x; 1.0953x over previous
"""Contrastive loss (NCE softmax over a similarity square) on 8 Trainium2 cores.

Math (B=8192, D=512, T=0.1, r=0.1):
    z   = normalize(emb)                       # row L2
    s   = sum_b emb[b, :]
    v_b = r*s + (1-2r)*emb[b];  pos_b = (z_b . v_b)/||v_b||
    loss = mean_b( log(S_b) - 10*pos_b )
    S_b = sum_j exp(10*raw[b,j]) + exp(10*pos_b) - e^10   (raw = z@z.T)

Sharding (v3): true data-parallel. Each core loads ONLY its own 1024-row
shard, normalizes it, transposes it (bf16 DMA transpose via DRAM), casts to
fp8e4 scaled by S=32, and AllGathers the fp8 zT blocks (0.5MB/rank -> 4MB).
The gathered layout [r][p][k][j] gives matmul-ready [K=128, N] tiles per
rank-block. Main loop: fp8 DoubleRow matmuls (2 k-subtiles per pass) into
[128,2048] psum, fused exp((10/S^2)*x) + row-sum accumulation on ACT.
The column-sum s = sum_b emb_b is computed per-shard via a tiny bf16
ones-matmul and AllReduced ([1,512] fp32). Host adds the 8 partial losses.
"""

import math

import numpy as np

import concourse.bacc as bacc
import concourse.mybir as mybir
import concourse.tile as tile
from concourse.bass_utils import run_bass_kernel_spmd

F32 = mybir.dt.float32
BF16 = mybir.dt.bfloat16
FP8 = mybir.dt.float8e4
AF = mybir.ActivationFunctionType
ALU = mybir.AluOpType
AX = mybir.AxisListType
DR = mybir.MatmulPerfMode.DoubleRow

B = 8192
D = 512
N_CORES = 8
OWN = B // N_CORES          # 1024 rows per core
P = 128                     # partitions
MT = OWN // P               # 8 own row tiles
KC = D // P                 # 4 contraction chunks of 128
NR = N_CORES                # 8 rank blocks of 1024 columns
SCALE = 10.0                # 1/TEMPERATURE
RATIO = 0.1
E10 = float(math.exp(SCALE))
S8 = 32.0                   # fp8 pre-scale; matmul result is S8^2 * sim
ESC = SCALE / (S8 * S8)     # exp scale folding the fp8 pre-scale back out


def _body(ctx, tc, out, emb_own):
    nc = tc.nc

    pp = ctx.enter_context(tc.tile_pool(name="persist", bufs=1))
    dp = ctx.enter_context(tc.tile_pool(name="dram", bufs=1, space="DRAM"))
    scrp = ctx.enter_context(tc.tile_pool(name="scrp", bufs=1))
    up = ctx.enter_context(tc.tile_pool(name="up", bufs=2))
    esp = ctx.enter_context(tc.tile_pool(name="esp", bufs=2))
    psm = ctx.enter_context(tc.tile_pool(name="psm", bufs=2, space="PSUM"))

    # persistent tiles
    eo = [pp.tile([P, D], F32, tag=f"eo_{m}", name=f"eo_{m}")
          for m in range(MT)]
    zof = [pp.tile([P, D], F32, tag=f"zof_{m}", name=f"zof_{m}")
           for m in range(MT)]
    zbf = [pp.tile([P, D], BF16, tag=f"zbf_{m}", name=f"zbf_{m}")
           for m in range(MT)]
    zTbf = [pp.tile([P, OWN], BF16, tag=f"zTbf_{k}", name=f"zTbf_{k}")
            for k in range(KC)]
    zT8 = pp.tile([P, KC * OWN], FP8, tag="zT8", name="zT8")
    zrh = [[pp.tile([P, KC * 512], FP8, tag=f"zr_{h}_{r}", name=f"zr_{h}_{r}")
            for r in range(NR)] for h in range(2)]
    osq = pp.tile([P, MT], F32, tag="osq", name="osq")
    oln = pp.tile([P, MT], F32, tag="oln", name="oln")
    oinv = pp.tile([P, MT], F32, tag="oinv", name="oinv")
    sinv = pp.tile([P, MT], F32, tag="sinv", name="sinv")
    normbf = pp.tile([P, MT], BF16, tag="normbf", name="normbf")
    scols = pp.tile([P, MT * 4], F32, tag="scols", name="scols")
    s01 = pp.tile([1, D], F32, tag="s01", name="s01")
    s01r = pp.tile([1, D], F32, tag="s01r", name="s01r")
    sbc = pp.tile([P, D], F32, tag="sbc", name="sbc")
    vsq = pp.tile([P, MT], F32, tag="vsq", name="vsq")
    zv = pp.tile([P, MT], F32, tag="zv", name="zv")
    vln = pp.tile([P, MT], F32, tag="vln", name="vln")
    vninv = pp.tile([P, MT], F32, tag="vninv", name="vninv")
    possim = pp.tile([P, MT], F32, tag="possim", name="possim")
    pos10 = pp.tile([P, MT], F32, tag="pos10", name="pos10")
    epos = pp.tile([P, MT], F32, tag="epos", name="epos")
    stot = pp.tile([P, MT], F32, tag="stot", name="stot")
    sfix = pp.tile([P, MT], F32, tag="sfix", name="sfix")
    lg = pp.tile([P, MT], F32, tag="lg", name="lg")
    loss8 = pp.tile([P, MT], F32, tag="loss8", name="loss8")
    ones_row = pp.tile([1, P], F32, tag="ones_row", name="ones_row")
    ones_col = pp.tile([P, 1], F32, tag="ones_col", name="ones_col")
    ones8 = pp.tile([MT, 1], F32, tag="ones8", name="ones8")
    l8 = pp.tile([MT, 1], F32, tag="l8", name="l8")
    res = pp.tile([1, 1], F32, tag="res", name="res")

    zodr = dp.tile([OWN, D], BF16, tag="zodr", name="zodr")
    ag_in = [dp.tile([P, KC * 512], FP8, tag=f"ag_in_{h}", name=f"ag_in_{h}")
             for h in range(2)]
    ar_in = dp.tile([1, D], F32, tag="ar_in", name="ar_in")
    ar_out = dp.tile([1, D], F32, tag="ar_out", name="ar_out")
    ag_out = [dp.tile([NR * P, KC * 512], FP8, tag=f"ag_out_{h}",
                      name=f"ag_out_{h}") for h in range(2)]

    nc.vector.memset(ones_row, 1.0)
    nc.vector.memset(ones_col, 1.0)
    nc.vector.memset(ones8, 1.0)

    # ---- Phase A: own shard -> zT8 (fp8, S8-scaled, [p][k][j] layout) ----
    for m in range(MT):
        eng = nc.sync if m % 2 == 0 else nc.scalar
        eng.dma_start(eo[m], emb_own[m * P:(m + 1) * P, :])
    for m in range(MT):
        scr = scrp.tile([P, D], F32, tag="scr", name="scr")
        nc.vector.scalar_tensor_tensor(
            out=scr, in0=eo[m], scalar=1.0, in1=eo[m],
            op0=ALU.mult, op1=ALU.mult, accum_out=osq[:, m:m + 1])
    # inv_norm = exp(-0.5*ln(x)); Ln+Exp stay within one ACT table set
    nc.scalar.activation(out=oln, in_=osq, func=AF.Ln)
    nc.scalar.activation(out=oinv, in_=oln, func=AF.Exp, scale=-0.5)
    nc.vector.tensor_scalar_mul(sinv, oinv, S8)
    # norm/S8 in bf16: lhsT for the s columns-sum matmul (s = sum_b emb_b)
    nc.vector.scalar_tensor_tensor(
        out=normbf, in0=osq, scalar=1.0 / S8, in1=oinv,
        op0=ALU.mult, op1=ALU.mult)
    # zodr writes and transposes share the sync queue: FIFO order is the
    # only guaranteed DRAM write->transpose-read ordering.
    for m in range(MT):
        nc.vector.tensor_scalar_mul(zbf[m], eo[m], sinv[:, m:m + 1])
        nc.sync.dma_start(zodr[m * P:(m + 1) * P, :], zbf[m])
    for m in range(MT):
        nc.vector.tensor_scalar_mul(zof[m], eo[m], oinv[:, m:m + 1])
    for k in range(KC):
        nc.sync.dma_start_transpose(zTbf[k], zodr[:, k * P:(k + 1) * P])
        nc.vector.tensor_copy(out=zT8[:, k * OWN:(k + 1) * OWN],
                              in_=zTbf[k])
    # sync queue: FIFO after the transposes; the AG's input-ready semaphore
    # is cross-queue tracked (proven by the ar_in path). Two j-halves so the
    # main loop can start after the first AllGather lands.
    zT8v3 = zT8.rearrange("p (k j) -> p k j", k=KC)
    for h in range(2):
        agv = ag_in[h].rearrange("p (k j) -> p k j", k=KC)
        nc.sync.dma_start(agv, zT8v3[:, :, h * 512:(h + 1) * 512])

    # s partial: sum_{own b} emb_b = sum_b (norm_b/S8) * (S8*z_b)  [bf16]
    s_ps = psm.tile([P, 2048], F32, tag="ps", name="ps_s")
    for m in range(MT):
        nc.tensor.matmul(s_ps[0:1, 0:D], lhsT=normbf[:, m:m + 1],
                         rhs=zbf[m], start=(m == 0), stop=(m == MT - 1),
                         skip_group_check=True)
    nc.vector.tensor_copy(out=s01, in_=s_ps[0:1, 0:D])
    nc.sync.dma_start(ar_in[:], s01)

    # ---- Collectives (gpsimd queue: AG1, h0 loads, AG2, AR, h1 loads) ----
    # zr loads stay on the gpsimd queue: FIFO-ordered after their AllGather.
    nc.gpsimd.collective_compute(
        "AllGather", ALU.bypass, replica_groups=[list(range(N_CORES))],
        ins=[ag_in[0].opt()], outs=[ag_out[0].opt()])
    for r in range(NR):
        nc.gpsimd.dma_start(zrh[0][r], ag_out[0][r * P:(r + 1) * P, :])
    nc.gpsimd.collective_compute(
        "AllGather", ALU.bypass, replica_groups=[list(range(N_CORES))],
        ins=[ag_in[1].opt()], outs=[ag_out[1].opt()])
    nc.gpsimd.collective_compute(
        "AllReduce", ALU.add, replica_groups=[list(range(N_CORES))],
        ins=[ar_in.opt()], outs=[ar_out.opt()])
    for r in range(NR):
        nc.gpsimd.dma_start(zrh[1][r], ag_out[1][r * P:(r + 1) * P, :])

    # ---- Main loop: 8192x8192/8 similarity slice, exp-sum fused ----
    zT8v = zT8v3
    zrv = [[zrh[h][r].rearrange("p (k j) -> p k j", k=KC) for r in range(NR)]
           for h in range(2)]

    def emit_pos_dve():
        # v = (1-2r)*emb + r*s (fp32, row-major); DVE-only, overlaps main
        nc.vector.tensor_scalar_mul(s01r, ar_dst, RATIO)
        ps_b = psm.tile([P, 2048], F32, tag="ps", name="ps_bc")
        nc.tensor.matmul(ps_b[:, 0:D], lhsT=ones_row, rhs=s01r,
                         start=True, stop=True, skip_group_check=True)
        nc.vector.tensor_copy(out=sbc, in_=ps_b[:, 0:D])
        for m in range(MT):
            u = up.tile([P, D], F32, tag="u", name="u")
            nc.vector.scalar_tensor_tensor(
                out=u, in0=eo[m], scalar=1.0 - 2.0 * RATIO, in1=sbc,
                op0=ALU.mult, op1=ALU.add)
            scr = scrp.tile([P, D], F32, tag="scr", name="scr")
            nc.vector.scalar_tensor_tensor(
                out=scr, in0=u, scalar=1.0, in1=u,
                op0=ALU.mult, op1=ALU.mult, accum_out=vsq[:, m:m + 1])
            scr2 = scrp.tile([P, D], F32, tag="scr", name="scr")
            nc.vector.scalar_tensor_tensor(
                out=scr2, in0=zof[m], scalar=1.0, in1=u,
                op0=ALU.mult, op1=ALU.mult, accum_out=zv[:, m:m + 1])

    ar_dst = pp.tile([1, D], F32, tag="ar_dst", name="ar_dst")
    for h in range(2):
        for rp in range(2):                 # r-block quartets within a half
            for m in range(MT):
                ps = psm.tile([P, 2048], F32, tag="ps", name="ps")
                for q in range(4):          # 4 r-blocks x 512-col half
                    r = rp * 4 + q
                    for kg in range(2):
                        nc.tensor.matmul(
                            ps[:, q * 512:(q + 1) * 512],
                            lhsT=zT8v[:, 2 * kg:2 * kg + 2,
                                      m * P:(m + 1) * P],
                            rhs=zrv[h][r][:, 2 * kg:2 * kg + 2, :],
                            start=(kg == 0), stop=(kg == 1),
                            perf_mode=DR, skip_group_check=True)
                es = esp.tile([P, 2048], BF16, tag="es", name="es")
                col = m * 4 + h * 2 + rp
                nc.scalar.activation(
                    out=es, in_=ps, func=AF.Exp, scale=ESC,
                    accum_out=scols[:, col:col + 1])
            if h == 1 and rp == 0:
                # pos-path ACT ops: by now vsq/zv are long done, and the
                # Ln<->Exp table switches hide in the ACT stream's slack
                nc.scalar.activation(out=vln, in_=vsq, func=AF.Ln)
                nc.scalar.activation(out=vninv, in_=vln, func=AF.Exp,
                                     scale=-0.5)
                nc.vector.tensor_mul(possim, zv, vninv)
                nc.vector.tensor_scalar_mul(pos10, possim, SCALE)
                nc.scalar.activation(out=epos, in_=pos10, func=AF.Exp)
        if h == 0:
            # AR lands between the halves; pos-path DVE work overlaps h=1
            nc.gpsimd.dma_start(ar_dst, ar_out[:])
            emit_pos_dve()

    # ---- Finale ----
    nc.vector.tensor_reduce(
        stot, scols.rearrange("p (m g) -> p m g", g=4), axis=AX.X,
        op=ALU.add)
    nc.vector.scalar_tensor_tensor(
        out=sfix, in0=stot, scalar=-E10, in1=epos, op0=ALU.add, op1=ALU.add)
    nc.scalar.activation(out=lg, in_=sfix, func=AF.Ln)
    nc.vector.tensor_sub(loss8, lg, pos10)
    f1 = psm.tile([P, 2048], F32, tag="ps", name="ps_f1")
    nc.tensor.matmul(f1[0:MT, 0:1], lhsT=loss8, rhs=ones_col,
                     start=True, stop=True, skip_group_check=True)
    nc.vector.tensor_copy(out=l8, in_=f1[0:MT, 0:1])
    f2 = psm.tile([P, 2048], F32, tag="ps", name="ps_f2")
    nc.tensor.matmul(f2[0:1, 0:1], lhsT=l8, rhs=ones8,
                     start=True, stop=True, skip_group_check=True)
    nc.vector.tensor_copy(out=res, in_=f2[0:1, 0:1])
    nc.sync.dma_start(out, res)


_NC_CACHE = None


def _build():
    global _NC_CACHE
    if _NC_CACHE is not None:
        return _NC_CACHE
    nc = bacc.Bacc(
        "TRN2",
        target_bir_lowering=False,
        debug=False,
        enable_asserts=False,
        num_devices=N_CORES,
    )
    emb_own = nc.dram_tensor("emb_own", [OWN, D], F32, kind="ExternalInput").ap()
    out = nc.dram_tensor("out", [1, 1], F32, kind="ExternalOutput").ap()
    from contextlib import ExitStack

    with tile.TileContext(nc) as tc, ExitStack() as ctx:
        _body(ctx, tc, out, emb_own)
    nc.compile()
    _NC_CACHE = nc
    return nc


def run(emb: np.ndarray, trace: bool = False):
    """Run the SPMD kernel; returns (loss, BassKernelResults)."""
    emb = np.ascontiguousarray(np.asarray(emb, dtype=np.float32))
    assert emb.shape == (B, D)
    nc = _build()
    in_maps = [
        {"emb_own": emb[c * OWN:(c + 1) * OWN]}
        for c in range(N_CORES)
    ]
    results = run_bass_kernel_spmd(
        nc, in_maps, core_ids=list(range(N_CORES)), trace=trace)
    total = 0.0
    for c in range(N_CORES):
        total += float(results.results[c]["out"][0, 0])
    loss = np.float32(total / B)
    return loss, results


def kernel(emb: np.ndarray) -> np.ndarray:
    loss, _ = run(emb, trace=False)
    return loss


if __name__ == "__main__":
    rng = np.random.default_rng(0)
    x = rng.standard_normal((B, D), dtype=np.float32)
    print("loss:", kernel(x))
